# revision 27
# baseline (speedup 1.0000x reference)
"""3-layer GCN (message passing) on 8 TRN2 NeuronCores.

Strategy: shard destination nodes across cores (graph parallel). Host
precomputes the graph structure (edge weights w = scale[type]*attr, degrees
incl. self-loop, dinv = rsqrt(deg)) and folds dinv_dst into the per-edge
weight. Per layer on device:
  h'_T = dinv * (prev @ W) computed on the node shard (PE + DVE),
  AllGather h' rows (bf16, split A/B so gather indices fit int16),
  per group of 2 dst blocks: dma_gather source rows on 4 SWDGE queues,
  build the weighted one-hot S_w[e,d] = (iota_d==dstloc[e])*w'[e] ON-CHIP
  (one dual-scalar-AP DVE tensor_scalar per 128-edge tile — no DRAM round
  trip), seed the PSUM accumulator with the self-loop term dinv^2*(hW) via
  an identity matmul, then accumulate out_T += M_e^T @ S_w on the PE.
  Epilogue is a single ACT Prelu (bias add + leaky) straight from PSUM.
The dense (h@W), dinv scaling, transposes and AllGather for layer l+1 are
interleaved into layer l's group loop at 512-col chunk granularity so the
collectives overlap the matmul tail and the PE stays warm (HAM 2.4GHz).
"""

import numpy as np

import concourse.bacc as bacc
import concourse.mybir as mybir
from concourse.tile import TileContext
from concourse.bass_utils import run_bass_kernel_spmd

try:
    import ml_dtypes

    BF16 = ml_dtypes.bfloat16
except ImportError:  # pragma: no cover
    BF16 = None

N_CORES = 8
D = 128
NEG_SLOPE = 0.1
G_BLOCKS = 2  # dst blocks per gather call group


def _ceil_div(a, b):
    return (a + b - 1) // b


def _wrap_idx(idx):
    """[cnt] int16 -> [128, cnt//16] wrapped layout (16-partition, replicated x8)."""
    cnt = idx.shape[0]
    assert cnt % 16 == 0
    w = idx.reshape(cnt // 16, 16).T  # [16, cnt//16]
    return np.tile(w, (8, 1)).astype(np.int16)  # [128, cnt//16]


def _preprocess(x, edge_index, edge_attr, edge_type, edge_type_scale):
    """Host-side sharding/layout. Returns (meta, per-core input arrays)."""
    N = x.shape[0]
    E = edge_index.shape[1]
    assert N % N_CORES == 0
    per = N // N_CORES
    nb = _ceil_div(per, 128)
    per_pad = nb * 128
    # split each core's shard rows at SA: half A rows [0,SA), half B [SA,per).
    SA = min(4096, (per // 2 // 128) * 128 + 1024)  # A fires early; keep B small
    assert SA % 128 == 0 and SA * N_CORES <= 32768

    src_f = np.asarray(edge_index[0], dtype=np.int64)
    dst_f = np.asarray(edge_index[1], dtype=np.int64)
    attr_f = np.asarray(edge_attr, dtype=np.float32)
    type_f = np.asarray(edge_type, dtype=np.int64)
    ets = np.asarray(edge_type_scale, dtype=np.float32)

    # per-edge weight, degree (self-loop adds +1), symmetric norm
    w_f = ets[type_f] * attr_f  # [E] f32
    deg = np.bincount(dst_f, weights=w_f.astype(np.float64), minlength=N)
    deg = deg + 1.0
    dinv = (1.0 / np.sqrt(deg)).astype(np.float32)  # [N]
    wp_f = w_f * dinv[dst_f]  # dinv_dst folded into edge weight

    core = dst_f // per
    ldst = dst_f - core * per
    blk = ldst >> 7
    slot = ldst & 127
    src_c = src_f // per
    src_r = src_f - src_c * per

    # rank-match per-core block sizes: slot b on every core holds its b-th
    # largest block (by edge count) so the maxed schedule has minimal padding
    perms = []
    tot_by_cb = np.zeros((N_CORES, nb), dtype=np.int64)
    for c in range(N_CORES):
        m = core == c
        tot_by_cb[c] = np.bincount(blk[m], minlength=nb)
    for c in range(N_CORES):
        perms.append(np.argsort(-tot_by_cb[c], kind="stable"))
    perms = np.asarray(perms)  # [C, nb]: schedule slot b <- local block perms[c][b]
    inv_perms = np.argsort(perms, axis=1)  # local block -> schedule slot

    SB = per_pad - SA
    assert SA % 128 == 0 and SB * N_CORES <= 32768
    # source rows in schedule coordinates of the source core
    src_rs = inv_perms[src_c, src_r >> 7] * 128 + (src_r & 127)
    half = (src_rs >= SA).astype(np.int64)
    gidx = np.where(half == 0, src_c * SA + src_rs, src_c * SB + (src_rs - SA))

    counts = np.zeros((N_CORES, nb, 2), dtype=np.int64)
    per_core = []
    for c in range(N_CORES):
        m = core == c
        s_src = src_f[m]
        s_blk = inv_perms[c][blk[m]]  # schedule-slot block index
        s_half = half[m]
        order = np.lexsort((s_src, s_half, s_blk))
        per_core.append(
            dict(
                src=gidx[m][order],
                blk=s_blk[order],
                slot=slot[m][order],
                wp=wp_f[m][order],
            )
        )
        cnt = np.bincount(s_blk * 2 + s_half, minlength=nb * 2).reshape(nb, 2)
        counts[c] = cnt

    # common padded schedule: tiles per (block, half), maxed over cores
    tiles_bh = np.maximum(1, _ceil_div(counts.max(axis=0), 128))  # [nb, 2]
    pad_bh = tiles_bh * 128

    groups = [list(range(g, min(g + G_BLOCKS, nb))) for g in range(0, nb, G_BLOCKS)]
    slot_off = np.zeros((nb, 2), dtype=np.int64)
    call_cnt = []  # per (g, half): total padded count
    off = 0
    for g in groups:
        for h in (0, 1):
            c0 = off
            for b in g:
                slot_off[b, h] = off
                off += pad_bh[b, h]
            call_cnt.append(off - c0)
    totslot = off
    T = totslot // 128

    tcols_b = []
    for b in range(nb):
        cols = list(range(slot_off[b, 0] // 128, slot_off[b, 0] // 128 + tiles_bh[b, 0]))
        cols += list(range(slot_off[b, 1] // 128, slot_off[b, 1] // 128 + tiles_bh[b, 1]))
        tcols_b.append(cols)

    ins = []
    for c in range(N_CORES):
        pc = per_core[c]
        idx_sl = np.zeros(totslot, dtype=np.int16)
        dst_sl = np.zeros(totslot, dtype=np.float32)
        wp_sl = np.zeros(totslot, dtype=np.float32)
        e0 = 0
        for b in range(nb):
            for h in (0, 1):
                n = counts[c, b, h]
                o = slot_off[b, h]
                if n:
                    sl = slice(e0, e0 + n)
                    idx_sl[o : o + n] = pc["src"][sl].astype(np.int16)
                    dst_sl[o : o + n] = pc["slot"][sl]
                    wp_sl[o : o + n] = pc["wp"][sl]
                    e0 += n

        wrapped = []
        off2 = 0
        for cc in call_cnt:
            wrapped.append(_wrap_idx(idx_sl[off2 : off2 + cc]))
            off2 += cc
        idx_w = np.concatenate(wrapped, axis=1)  # [128, totslot//16]

        col = lambda a: np.ascontiguousarray(a.reshape(T, 128).T)  # [128, T]
        xt = np.zeros((128, per_pad), dtype=BF16)
        dinvr = np.zeros((1, per_pad), dtype=np.float32)
        xs = np.asarray(x[c * per : (c + 1) * per], dtype=np.float32)
        ds = dinv[c * per : (c + 1) * per]
        for b in range(nb):
            lb = int(perms[c][b])
            n = min(128, per - lb * 128)
            xt[:, b * 128 : b * 128 + n] = xs[lb * 128 : lb * 128 + n].T.astype(BF16)
            dinvr[0, b * 128 : b * 128 + n] = ds[lb * 128 : lb * 128 + n]
        ins.append(
            dict(
                IDX=idx_w,
                DSTLOC=col(dst_sl).astype(BF16),
                WCOL=col(wp_sl).astype(BF16),
                XT=xt,
                DINVR=dinvr,
            )
        )

    meta = dict(
        N=N, E=E, per=per, nb=nb, per_pad=per_pad, SA=SA, T=T,
        totslot=totslot, groups=groups, call_cnt=call_cnt, tiles_bh=tiles_bh,
        slot_off=slot_off, tcols_b=tcols_b, perms=perms,
    )
    return meta, ins


def _build(meta):
    per = meta["per"]
    nb = meta["nb"]
    per_pad = meta["per_pad"]
    SA = meta["SA"]
    SB = per_pad - SA
    T = meta["T"]
    totslot = meta["totslot"]
    groups = meta["groups"]
    call_cnt = meta["call_cnt"]
    tiles_bh = meta["tiles_bh"]
    tcols_b = meta["tcols_b"]

    f32 = mybir.dt.float32
    bf16 = mybir.dt.bfloat16
    i16 = mybir.dt.int16

    maxw128 = max(c // 128 for c in call_cnt)
    maxw_h = [
        max(c // 128 for c in call_cnt[0::2]),
        max(c // 128 for c in call_cnt[1::2]),
    ]
    call_base = [sum(call_cnt[:i]) for i in range(len(call_cnt))]

    nc = bacc.Bacc("TRN2", num_devices=N_CORES, num_swdge_queues=4,
                   dynamic_dma_scratch_size=65536)

    t_idx = nc.dram_tensor("IDX", [128, totslot // 16], i16, kind="ExternalInput")
    t_dstloc = nc.dram_tensor("DSTLOC", [128, T], bf16, kind="ExternalInput")
    t_wcol = nc.dram_tensor("WCOL", [128, T], bf16, kind="ExternalInput")
    t_iota_b = nc.dram_tensor("IOTAB", [128, 128], bf16, kind="ExternalInput")
    t_xt = nc.dram_tensor("XT", [128, per_pad], bf16, kind="ExternalInput")
    t_dinvr = nc.dram_tensor("DINVR", [1, per_pad], f32, kind="ExternalInput")
    t_W = [
        nc.dram_tensor(f"W{i}", [128, 128], bf16, kind="ExternalInput")
        for i in (1, 2, 3)
    ]
    t_b = [
        nc.dram_tensor(f"b{i}", [128, 1], f32, kind="ExternalInput") for i in (1, 2, 3)
    ]
    t_ident = nc.dram_tensor("IDENT", [128, 128], f32, kind="ExternalInput")
    t_identb = nc.dram_tensor("IDENTB", [128, 128], bf16, kind="ExternalInput")
    t_ones_r = nc.dram_tensor("ONESR", [1, 128], f32, kind="ExternalInput")
    t_out = nc.dram_tensor("OUT", [per_pad, 128], f32, kind="ExternalOutput")

    hcurA = [
        nc.dram_tensor(f"hcurA{l}", [SA, 128], bf16, kind="Internal") for l in range(3)
    ]
    hcurB = [
        nc.dram_tensor(f"hcurB{l}", [SB, 128], bf16, kind="Internal") for l in range(3)
    ]
    hfullA = [
        nc.dram_tensor(
            f"hfullA{l}", [N_CORES * SA, 128], bf16, kind="Internal",
            addr_space="Shared",
        )
        for l in range(3)
    ]
    hfullB = [
        nc.dram_tensor(
            f"hfullB{l}", [N_CORES * SB, 128], bf16, kind="Internal",
            addr_space="Shared",
        )
        for l in range(3)
    ]
    rg = [list(range(N_CORES))]

    def chunks512(total):
        out = []
        o = 0
        while o < total:
            w = min(512, total - o)
            out.append((o, w))
            o += w
        return out

    with TileContext(nc) as tc:
        with (
            tc.tile_pool(name="persist", bufs=1) as pp,
            tc.tile_pool(name="work", bufs=2) as wp,
            tc.tile_pool(name="mtp", bufs=6) as mtp,
            tc.tile_pool(name="swp", bufs=2) as swp,
            tc.tile_pool(name="psum", bufs=2, space="PSUM") as psp,
            tc.tile_pool(name="psumt", bufs=2, space="PSUM") as pst,
            tc.tile_pool(name="psumg", bufs=4, space="PSUM") as pspg,
        ):
            qsem = [nc.alloc_semaphore(f"swdge_dma_q{q}") for q in range(4)]

            # ---------- persistent loads ----------
            DSTLOC = pp.tile([128, T], bf16, tag="DSTLOC")
            nc.sync.dma_start(DSTLOC[:, :], t_dstloc[:, :])
            WCOL = pp.tile([128, T], bf16, tag="WCOL")
            nc.sync.dma_start(WCOL[:, :], t_wcol[:, :])
            IOTAB = pp.tile([128, 128], bf16, tag="IOTAB")
            nc.sync.dma_start(IOTAB[:, :], t_iota_b[:, :])
            IDENT = pp.tile([128, 128], f32, tag="IDENT")
            nc.sync.dma_start(IDENT[:, :], t_ident[:, :])
            IDENTB = pp.tile([128, 128], bf16, tag="IDENTB")
            nc.sync.dma_start(IDENTB[:, :], t_identb[:, :])
            ONESR = pp.tile([1, 128], f32, tag="ONESR")
            nc.sync.dma_start(ONESR[:, :], t_ones_r[:, :])
            W = []
            B = []
            for i in range(3):
                Wt = pp.tile([128, 128], bf16, tag=f"W{i}")
                nc.sync.dma_start(Wt[:, :], t_W[i][:, :])
                W.append(Wt)
                Bt = pp.tile([128, 1], f32, tag=f"B{i}")
                nc.sync.dma_start(Bt[:, :], t_b[i][:, :])
                B.append(Bt)

            DINVB = pp.tile([128, per_pad], bf16, tag="DINVB")
            # double-buffered feature tables (bf16): current / next layer
            HP1 = pp.tile([128, per_pad], bf16, tag="HP1", name="HP1")
            HP = [HP1, HP1]
            HP2 = [
                pp.tile([128, per_pad], bf16, tag=f"HP2{i}", name=f"HP2{i}")
                for i in range(2)
            ]
            HOUT = pp.tile([128, per_pad], bf16, tag="HOUT")

            # ---------- DINVB = broadcast(dinv) via rank-1 outer product ----
            for o, cw in chunks512(per_pad):
                dvr = wp.tile([1, 512], f32, tag="dvr")
                nc.sync.dma_start(dvr[:, :cw], t_dinvr[:, o : o + cw])
                pb = psp.tile([128, 512], f32, tag="p512")
                nc.tensor.matmul(
                    pb[:, :cw], ONESR[:, :], dvr[0:1, :cw],
                    start=True, stop=True,
                )
                nc.vector.tensor_copy(DINVB[:, o : o + cw], pb[:, :cw])

            def table_chunk(lt, o, cw, ph):
                """HP[lt] = dinv * ph, HP2[lt] = dinv * HP[lt] for chunk cols."""
                nc.vector.tensor_tensor(
                    HP[lt][:, o : o + cw], ph[:, :cw], DINVB[:, o : o + cw],
                    op=mybir.AluOpType.mult,
                )
                nc.vector.tensor_tensor(
                    HP2[lt][:, o : o + cw], HP[lt][:, o : o + cw],
                    DINVB[:, o : o + cw], op=mybir.AluOpType.mult,
                )

            def transpose_store(lt, l, cb):
                """Transpose HP[lt][:, block cb] to rows; store into hcurA/B[l]."""
                pt = pst.tile([128, 128], bf16, tag="ptr")
                nc.tensor.transpose(
                    pt[:, :], HP[lt][:, cb * 128 : (cb + 1) * 128], IDENTB[:, :]
                )
                rt = wp.tile([128, 128], bf16, tag="rowb", bufs=4)
                nc.vector.tensor_copy(rt[:, :], pt[:, :])
                r0 = cb * 128
                r1 = r0 + 128
                if r1 <= SA:
                    nc.sync.dma_start(hcurA[l][r0:r1, :], rt[0 : r1 - r0, :])
                elif r0 >= SA:
                    nc.sync.dma_start(
                        hcurB[l][r0 - SA : r1 - SA, :], rt[0 : r1 - r0, :]
                    )
                else:
                    nc.sync.dma_start(hcurA[l][r0:SA, :], rt[0 : SA - r0, :])
                    nc.sync.dma_start(
                        hcurB[l][0 : r1 - SA, :], rt[SA - r0 : r1 - r0, :]
                    )

            def fire_ag(l, which):
                if which == 0:
                    nc.gpsimd.collective_compute(
                        "AllGather", mybir.AluOpType.bypass,
                        ins=[hcurA[l][:, :]], outs=[hfullA[l][:, :]],
                        replica_groups=rg,
                    )
                else:
                    nc.gpsimd.collective_compute(
                        "AllGather", mybir.AluOpType.bypass,
                        ins=[hcurB[l][:, :]], outs=[hfullB[l][:, :]],
                        replica_groups=rg,
                    )

            # ---------- bootstrap: table 0 = dinv*(x@W1) ----------
            sa_blocks = _ceil_div(SA, 128)  # AG-A ready once blocks 0..sa_blocks-1 stored
            ag_fired = [False, False]
            for ci_, (o, cw) in enumerate(chunks512(per_pad)):
                xc = wp.tile([128, 512], bf16, tag="xc")
                nc.sync.dma_start(xc[:, :cw], t_xt[:, o : o + cw])
                ph = psp.tile([128, 512], f32, tag="p512")
                nc.tensor.matmul(ph[:, :cw], W[0][:, :], xc[:, :cw], start=True, stop=True)
                table_chunk(0, o, cw, ph)
                for cb in range(o // 128, (o + cw) // 128):
                    transpose_store(0, 0, cb)
                if not ag_fired[0] and (o + cw) // 128 >= sa_blocks:
                    fire_ag(0, 0)
                    ag_fired[0] = True
            fire_ag(0, 1)

            # ---------- layers ----------
            for l in range(3):
                lt = l % 2  # current table parity
                nt_ = (l + 1) % 2  # next table parity
                ag_fired = [False, False]
                done_chunks = 0  # 512-col chunks of HOUT consumed for next layer
                n_chunks = len(chunks512(per_pad))

                def flush_next_layer(upto_col):
                    """Dense + table + transpose + AG for layer l+1 using HOUT
                    columns < upto_col (in 512 chunks)."""
                    nonlocal done_chunks
                    if l >= 2:
                        return
                    ch = chunks512(per_pad)
                    while done_chunks < n_chunks and (
                        ch[done_chunks][0] + ch[done_chunks][1] <= upto_col
                    ):
                        o, cw = ch[done_chunks]
                        ph = psp.tile([128, 512], f32, tag="p512")
                        nc.tensor.matmul(
                            ph[:, :cw], W[l + 1][:, :], HOUT[:, o : o + cw],
                            start=True, stop=True,
                        )
                        table_chunk(nt_, o, cw, ph)
                        for cb in range(o // 128, (o + cw) // 128):
                            transpose_store(nt_, l + 1, cb)
                        done_chunks += 1
                        if not ag_fired[0] and done_chunks * 4 >= sa_blocks:
                            fire_ag(l + 1, 0)
                            ag_fired[0] = True

                TRIG_LAG = 4
                maxw16 = max(c_ // 16 for c_ in call_cnt)
                group_mts = {}
                pending_q = [0, 0, 0, 0]

                def prep_group(gi2):
                    mts2 = {}
                    for h in (0, 1):
                        ci = 2 * gi2 + h
                        cnt = call_cnt[ci]
                        woff = call_base[ci] // 16
                        idxt = wp.tile([128, maxw16], i16, tag="idx", bufs=8)
                        nc.sync.dma_start(
                            idxt[:, : cnt // 16], t_idx[:, woff : woff + cnt // 16]
                        )
                        mt = mtp.tile(
                            [128, maxw_h[h], 128], bf16, tag=f"m{h}", name=f"mt{h}"
                        )
                        src_tab = hfullA[l][:, :] if h == 0 else hfullB[l][:, :]
                        nc.gpsimd.dma_gather(
                            mt[:, : cnt // 128, :], src_tab,
                            idxt[:, : cnt // 16],
                            num_idxs=cnt, num_idxs_reg=cnt, elem_size=128,
                            single_packet=False, queue_num=ci % 4,
                        )
                        mts2[h] = mt
                    group_mts[gi2] = mts2

                def trig_group(gi2):
                    pass

                for gi2 in range(min(TRIG_LAG, len(groups))):
                    prep_group(gi2)

                for gi, g in enumerate(groups):
                    gw = len(g) * 128
                    g0 = g[0] * 128
                    trig_group(gi)
                    if gi + TRIG_LAG < len(groups):
                        prep_group(gi + TRIG_LAG)
                    mts = group_mts.pop(gi)
                    # on-chip weighted one-hot: S_w[e, d] = (iota_d==dst_e)*w'_e
                    gbase = call_base[2 * gi] // 128
                    gtiles = (call_cnt[2 * gi] + call_cnt[2 * gi + 1]) // 128
                    swg = swp.tile([128, maxw_h[0] + maxw_h[1], 128], bf16, tag="swg")
                    ib = IOTAB[:, :].unsqueeze(1).to_broadcast([128, gtiles, 128])
                    db = (
                        DSTLOC[:, gbase : gbase + gtiles]
                        .unsqueeze(2)
                        .to_broadcast([128, gtiles, 128])
                    )
                    nc.vector.tensor_tensor(
                        swg[:, :gtiles, :], ib, db, op=mybir.AluOpType.is_equal
                    )
                    wb = (
                        WCOL[:, gbase : gbase + gtiles]
                        .unsqueeze(2)
                        .to_broadcast([128, gtiles, 128])
                    )
                    nc.vector.tensor_tensor(
                        swg[:, :gtiles, :], swg[:, :gtiles, :], wb,
                        op=mybir.AluOpType.mult,
                    )

                    pg = pspg.tile([128, G_BLOCKS * 128], f32, tag="pblk")
                    # seed with self-loop term dinv^2*(hW) via identity matmul
                    nc.tensor.matmul(
                        pg[:, :gw], IDENTB[:, :], HP2[lt][:, g0 : g0 + gw],
                        start=True, stop=False,
                    )
                    for bi, b in enumerate(g):
                        cols = tcols_b[b]
                        nlo = int(tiles_bh[b][0])
                        for ti, tcol in enumerate(cols):
                            hh = 0 if ti < nlo else 1
                            j = tcol - call_base[2 * gi + hh] // 128
                            nc.tensor.matmul(
                                pg[:, bi * 128 : (bi + 1) * 128],
                                mts[hh][:, j, :], swg[:, tcol - gbase, :],
                                start=False, stop=(ti == len(cols) - 1),
                            )
                    # epilogue: bias + leaky (or plain bias on the last layer)
                    if l < 2:
                        nc.scalar.activation(
                            HOUT[:, g0 : g0 + gw], pg[:, :gw],
                            mybir.ActivationFunctionType.Prelu,
                            bias=B[l][:, 0:1], scale=1.0, alpha=NEG_SLOPE,
                        )
                    else:
                        fo = wp.tile([128, G_BLOCKS * 128], f32, tag="fout", bufs=3)
                        nc.scalar.activation(
                            fo[:, :gw], pg[:, :gw],
                            mybir.ActivationFunctionType.Identity,
                            bias=B[l][:, 0:1], scale=1.0,
                        )
                        for bi, b in enumerate(g):
                            pt = pst.tile([128, 128], f32, tag="ptr")
                            nc.tensor.transpose(
                                pt[:, :], fo[:, bi * 128 : (bi + 1) * 128],
                                IDENT[:, :],
                            )
                            rf = wp.tile([128, 128], f32, tag="rowf", bufs=3)
                            nc.vector.tensor_copy(rf[:, :], pt[:, :])
                            r0 = b * 128
                            nc.sync.dma_start(t_out[r0 : r0 + 128, :], rf[:, :])
                    flush_next_layer(g0 + gw)

                if l < 2:
                    flush_next_layer(per_pad)
                    fire_ag(l + 1, 1)

    nc.compile()
    return nc


_CACHE = {}


def kernel(
    x,
    edge_index,
    edge_attr,
    edge_type,
    edge_type_scale,
    W1,
    b1,
    W2,
    b2,
    W3,
    b3,
):
    x = np.asarray(x)
    N = x.shape[0]
    meta, per_core = _preprocess(
        np.asarray(x), np.asarray(edge_index), np.asarray(edge_attr),
        np.asarray(edge_type), np.asarray(edge_type_scale),
    )

    key = (N, meta["T"], tuple(meta["call_cnt"]))
    if key not in _CACHE:
        _CACHE[key] = _build(meta)
    nc = _CACHE[key]

    ident = np.eye(128, dtype=np.float32)
    iota_f = np.tile(np.arange(128, dtype=np.float32)[None, :], (128, 1))
    common = dict(
        IOTAB=iota_f.astype(BF16),
        W1=np.asarray(W1, np.float32).astype(BF16),
        W2=np.asarray(W2, np.float32).astype(BF16),
        W3=np.asarray(W3, np.float32).astype(BF16),
        b1=np.asarray(b1, np.float32).reshape(D, 1),
        b2=np.asarray(b2, np.float32).reshape(D, 1),
        b3=np.asarray(b3, np.float32).reshape(D, 1),
        IDENT=ident,
        IDENTB=ident.astype(BF16),
        ONESR=np.ones((1, 128), np.float32),
    )
    in_maps = []
    for c in range(N_CORES):
        m = dict(common)
        m["IDX"] = per_core[c]["IDX"]
        m["DSTLOC"] = per_core[c]["DSTLOC"]
        m["WCOL"] = per_core[c]["WCOL"]
        m["XT"] = per_core[c]["XT"]
        m["DINVR"] = per_core[c]["DINVR"]
        in_maps.append(m)

    res = run_bass_kernel_spmd(
        nc, in_maps, core_ids=list(range(N_CORES)), **_RUN_KWARGS
    )
    _LAST_RESULT.clear()
    _LAST_RESULT["exec_time_ns"] = res.exec_time_ns
    _LAST_RESULT["profile_json"] = res.profile_json
    per = meta["per"]
    nb_ = (per + 127) // 128
    perms = meta["perms"]
    outs = []
    for c in range(N_CORES):
        o = np.asarray(res.results[c]["OUT"])  # [per_pad, 128] schedule order
        op = np.zeros((per, o.shape[1]), o.dtype)
        for b in range(nb_):
            lb = int(perms[c][b])  # schedule slot b holds local block lb
            n = min(128, per - lb * 128)
            op[lb * 128 : lb * 128 + n] = o[b * 128 : b * 128 + n]
        outs.append(op)
    out = np.concatenate(outs, axis=0)
    return out.astype(np.float32)


_RUN_KWARGS = {}  # test harness can set {"trace": True, "tmpdir": ...}
_LAST_RESULT = {}
